# revision 43
# baseline (speedup 1.0000x reference)
"""Trainium2 Bass kernel for nn_ColorExtractor (per-image k-means, K=8, 10 iters).

Contract: kernel(**inputs) takes the FULL inputs (inputs: [64, 512, 512, 3] f32)
and returns the FULL output ([64, 24] f32), sharding batch across 8 NeuronCores.

Device algorithm per image (N = 262144 pixels):
  scores  s_k = |x|^2 - 2 c_k . x + |c_k|^2  (TRUE squared distance) via one
          f32r PE matmul per group of 4096 pixels (xT carries a 4th |x|^2
          band; block-diagonal Wdiag over j, k-major columns) plus a tiny
          contract-1 f32r matmul accumulating |c_k|^2. f32r runs at 1
          cycle/row for free dim >= 256 (vs 4 for plain fp32) with ~2^-13
          accuracy (measured on HW) -- no bf16 hi/lo split needed. Using true
          d^2 >= 0 keeps the decisive (near-min) score range tiny, so the
          fp16 conversion below does not perturb the argmin.
  argmin  scores are converted f32(PSUM) -> fp16(SBUF) on the Activation
          engine; DVE then computes the per-pixel min with a 3-level
          pairwise-min tree over the k-major layout and the 0/1 masks with a
          single is_le against the broadcast min. All DVE operands are 2-byte
          and packed, so every op runs in the DVE 2x perf mode.
  update  segment sums via PE "diagonal-block" matmul (bf16 x, bf16 masks):
          seg[(c,j1), (k,j2)] += sum_p xpix[p, (c,j1)] * mask[p, (k,j2)]
          diagonal j1==j2 blocks hold per-chunk per-cluster sums of (r,g,b,1);
          extracted with a 0/1 diag mask + reduce, then a tiny matmul folds
          chunks -> S[k, (r,g,b,count)]; empty clusters keep old centroid.
          Segsum matmuls are emitted one batch behind the scores matmuls so
          the PE never stalls on the DVE min/mask chain.

Initial centroids replicate jax.random.permutation(key, N)[:8] (threefry,
stock-jax CPU semantics) via a pure-numpy threefry port embedded below.
"""

import numpy as np

import concourse.bacc as bacc
import concourse.bass as bass
import concourse.tile as tile
from concourse import mybir
from concourse.bass_utils import run_bass_kernel_spmd

# ----------------------------------------------------------------------------
# problem constants (hardcoded per contract)
B = 64            # total images
NCORES = 8
IMG_PER_CORE = B // NCORES
H = W = 512
N = H * W         # pixels per image: 262144
K = 8             # clusters
ITERS = 10
D = 3

# device tiling
P = 128           # pixels per chunk (contract dim of segsum matmul)
J = 32            # chunks per matmul group
GROUPS = N // (J * P)   # 64 matmul groups per image
GBATCH = 4        # groups per PSUM batch for the DVE min/mask ops
FD = J * K        # 256: free dim of the scores / segsum matmuls

F32 = mybir.dt.float32
F32R = mybir.dt.float32r
BF16 = mybir.dt.bfloat16
F16 = mybir.dt.float16

# ----------------------------------------------------------------------------
# numpy threefry port (verified bit-exact vs jax 0.8 threefry2x32 impl)
_U32 = np.uint32


def _rotl(x, d):
    d = _U32(d)
    return (x << d) | (x >> _U32(32 - d))


def _threefry2x32(k1, k2, x1, x2):
    with np.errstate(over="ignore"):
        ks0, ks1 = _U32(k1), _U32(k2)
        ks2 = _U32(ks0 ^ ks1 ^ _U32(0x1BD11BDA))
        x = [(x1 + ks0).astype(_U32), (x2 + ks1).astype(_U32)]

        def rounds(rots, ka, kb, inc):
            for r in rots:
                x[0] = (x[0] + x[1]).astype(_U32)
                x[1] = _rotl(x[1], r)
                x[1] = x[0] ^ x[1]
            x[0] = (x[0] + ka).astype(_U32)
            x[1] = (x[1] + kb + _U32(inc)).astype(_U32)

        rounds((13, 15, 26, 6), ks1, ks2, 1)
        rounds((17, 29, 16, 24), ks2, ks0, 2)
        rounds((13, 15, 26, 6), ks0, ks1, 3)
        rounds((17, 29, 16, 24), ks1, ks2, 4)
        rounds((13, 15, 26, 6), ks2, ks0, 5)
    return x[0], x[1]


def _tf_split(key, num):
    i = np.arange(num, dtype=np.uint64)
    b1, b2 = _threefry2x32(key[0], key[1],
                           (i >> np.uint64(32)).astype(_U32), i.astype(_U32))
    return np.stack([b1, b2], axis=1)


def _tf_bits(key, n):
    i = np.arange(n, dtype=np.uint64)
    b1, b2 = _threefry2x32(key[0], key[1],
                           (i >> np.uint64(32)).astype(_U32), i.astype(_U32))
    return b1 ^ b2


def jax_permutation_indices(seed, batch, n):
    """perm[b] = jax.random.permutation(split(key(seed), batch)[b], n)."""
    keys = _tf_split(np.array([0, seed], _U32), batch)
    num_rounds = int(np.ceil(3 * np.log(max(1, n)) / np.log(2**32 - 1)))
    perms = []
    for b in range(batch):
        x = np.arange(n)
        k = keys[b]
        for _ in range(num_rounds):
            ks = _tf_split(k, 2)
            k = ks[0]
            sort_keys = _tf_bits(ks[1], n)
            x = x[np.argsort(sort_keys, kind="stable")]
        perms.append(x[:K])
    return np.stack(perms)  # [batch, K]


# Precomputed jax.random.permutation(split(key(42), 64)[b], N)[:8] indices
# (input-independent; verified against the threefry port above).
PERM8 = (
    (121373, 128858, 64733, 199519, 198377, 234239, 198325, 209106),
    (73520, 236184, 209288, 97370, 64322, 228694, 126128, 72161),
    (143944, 27877, 97040, 2149, 10994, 109181, 179954, 54887),
    (147613, 8773, 54262, 44295, 29289, 11407, 31612, 133442),
    (206432, 166428, 5023, 212109, 16365, 21194, 249053, 195143),
    (13257, 110295, 84080, 119151, 246640, 69532, 130091, 105945),
    (14760, 174397, 198857, 826, 140745, 258776, 214608, 163989),
    (184593, 240934, 160738, 23779, 43199, 47433, 94941, 50416),
    (4386, 21260, 129661, 125128, 50701, 200388, 254109, 44816),
    (203980, 230711, 102351, 31296, 161690, 63692, 194032, 60281),
    (170168, 75997, 12072, 137876, 34146, 48636, 181597, 67859),
    (218987, 48148, 224774, 27163, 85280, 163529, 107708, 238871),
    (152153, 120028, 50368, 168498, 254864, 185234, 259971, 5221),
    (126051, 57270, 7614, 194865, 246341, 83824, 226962, 115962),
    (68603, 18235, 201699, 6558, 217064, 74053, 140307, 29320),
    (212222, 174163, 63891, 131714, 260991, 125525, 109871, 254552),
    (208133, 37817, 108871, 236086, 230829, 224735, 197202, 126789),
    (36220, 183667, 173531, 231574, 63007, 23270, 242256, 172824),
    (226174, 181177, 45094, 10219, 172720, 14537, 122494, 27364),
    (19288, 1130, 162371, 12239, 106820, 190833, 228451, 33845),
    (420, 256427, 250298, 234965, 137965, 33886, 192615, 137263),
    (30426, 206099, 1480, 169907, 122972, 5299, 178194, 116853),
    (38366, 252943, 119579, 233642, 99176, 152381, 1818, 246484),
    (49412, 124354, 252000, 221213, 103625, 2726, 153653, 148581),
    (82319, 1626, 107383, 158105, 81846, 13120, 1198, 193305),
    (44406, 239081, 240884, 84662, 7763, 52627, 182256, 187716),
    (185632, 105456, 212756, 173585, 81328, 74972, 128159, 45046),
    (104599, 7215, 61087, 26573, 59314, 48591, 945, 28553),
    (127710, 94893, 75476, 221733, 184125, 96685, 172243, 242612),
    (42647, 29769, 148111, 39823, 193859, 57502, 144317, 214559),
    (780, 145567, 79710, 226978, 2835, 160638, 8378, 24523),
    (161231, 246284, 44873, 150516, 114149, 68239, 117811, 141424),
    (31461, 110744, 232951, 16033, 179041, 106854, 47200, 63782),
    (255322, 241469, 248608, 95048, 170033, 253394, 261582, 181885),
    (63034, 5, 212309, 79222, 1841, 237107, 261430, 22474),
    (203738, 21095, 211942, 6233, 26825, 175918, 126433, 89713),
    (57893, 173681, 13566, 126980, 140303, 73406, 105028, 86705),
    (15800, 76765, 217596, 184873, 201602, 112166, 76158, 112065),
    (110522, 160113, 18684, 10469, 166599, 145226, 99589, 158310),
    (214726, 131223, 109288, 126812, 105792, 167086, 256918, 18441),
    (164736, 182565, 35066, 89660, 98586, 130539, 202194, 16684),
    (24903, 25959, 122313, 26525, 105627, 87218, 23062, 109362),
    (67552, 140412, 247510, 126439, 184322, 171107, 87397, 165128),
    (211326, 162921, 221946, 131793, 156106, 253917, 2345, 133918),
    (219591, 25610, 154884, 239521, 173390, 39973, 114213, 162088),
    (69694, 51180, 74827, 176121, 132947, 148345, 15083, 196459),
    (229624, 100015, 196100, 105569, 78527, 72176, 225549, 208691),
    (158498, 42753, 240006, 246065, 213196, 49877, 129372, 244273),
    (51001, 229538, 39704, 237637, 58774, 83576, 211231, 135814),
    (173630, 162748, 219633, 240928, 8298, 5311, 113776, 113251),
    (64061, 16436, 138070, 47525, 57016, 229742, 159929, 228539),
    (73108, 34503, 7538, 165920, 68681, 114191, 193009, 48042),
    (2842, 97501, 29489, 248778, 176907, 223147, 54452, 11731),
    (224345, 79068, 183290, 239324, 14912, 169078, 122283, 32914),
    (95340, 11646, 45163, 48387, 78062, 60978, 227735, 162106),
    (258986, 131616, 85766, 51383, 132449, 213013, 150516, 231609),
    (65332, 246689, 206208, 181886, 235636, 139183, 132468, 6602),
    (6778, 179487, 58159, 114248, 26277, 180706, 54969, 240497),
    (15413, 19595, 73952, 219244, 68813, 152629, 243501, 175077),
    (208668, 251169, 186627, 98857, 78225, 13125, 12392, 28954),
    (81754, 93281, 49839, 112579, 166016, 88571, 91558, 20863),
    (108264, 245898, 72992, 168504, 68263, 195879, 27596, 23576),
    (44918, 166098, 212537, 239555, 231283, 94408, 203172, 18701),
    (113563, 111669, 16481, 161974, 22111, 116384, 31096, 252828),
)


# ----------------------------------------------------------------------------
# device kernel builder


def build_kernel(n_img=IMG_PER_CORE, iters=ITERS, groups=GROUPS):
    assert n_img % 2 == 0, "pair-interleaved kernel needs an even image count"
    n_pix = groups * J * P
    nc = bacc.Bacc("TRN2", target_bir_lowering=False)

    # xplanar holds the xT SBUF layout directly: [img, (c, j), (g, p)]
    # with a 4th band c=3 carrying |x|^2 (host-computed)
    xpl = nc.dram_tensor("xplanar", [n_img, 4 * J, n_pix // J], F32,
                         kind="ExternalInput")
    c0 = nc.dram_tensor("cent0", [n_img, K, D], F32, kind="ExternalInput")
    diag01_d = nc.dram_tensor("diag01", [P, FD], F32, kind="ExternalInput")
    csel_d = nc.dram_tensor("csel", [P, 4], F32, kind="ExternalInput")
    ident_d = nc.dram_tensor("ident", [P, P], F32, kind="ExternalInput")
    out_d = nc.dram_tensor("cent_out", [n_img, K, D], F32, kind="ExternalOutput")

    NBUF = 4          # xT buffers: 2 per active pair + 2 prefetch
    with tile.TileContext(nc) as tc:
        with (
            tc.tile_pool(name="singles", bufs=1) as singles,
            tc.tile_pool(name="masks", bufs=4) as maskpool,
            tc.tile_pool(name="s16", bufs=4) as s16pool,
            tc.tile_pool(name="mvals", bufs=4) as mvpool,
            tc.tile_pool(name="smallsb", bufs=2) as smallsb,
            tc.tile_pool(name="bigpsum", bufs=2, space="PSUM") as bigpsum,
            tc.tile_pool(name="segpsum", bufs=1, space="PSUM") as segpsum,
            tc.tile_pool(name="smallpsum", bufs=1, space="PSUM") as smallps,
        ):
            dumpps = smallps
            # --- constants ---
            # Constants consumed by matmuls are staged through one DVE copy so
            # matmul waits collapse onto the DVE semaphore (fp32 matmuls have
            # only 2 sync-wait slots).
            diag01 = singles.tile([P, FD], F32, tag="diag01")
            nc.sync.dma_start(out=diag01[:], in_=diag01_d[:])
            csel_l = singles.tile([P, 4], F32, tag="csel_l")
            nc.sync.dma_start(out=csel_l[:], in_=csel_d[:])
            ident_l = singles.tile([P, P], F32, tag="ident_l")
            nc.sync.dma_start(out=ident_l[:], in_=ident_d[:])
            csel = singles.tile([P, 4], F32, tag="csel")
            nc.vector.tensor_copy(csel[:], csel_l[:])
            ident = singles.tile([P, P], F32, tag="ident")
            nc.vector.tensor_copy(ident[:], ident_l[:])

            # --- persistent state ---
            # xT bands: c=0..2 pixel colors, c=3 |x|^2 (from host); f32r via
            # casting gpsimd DMA (walrus requires f32r-rounded producers)
            xT = [singles.tile([P, groups * P], F32R, tag=f"xT{i}",
                               name=f"xT{i}") for i in range(NBUF)]
            identr = singles.tile([P, P], F32R, tag="identr")
            nc.vector.tensor_copy(identr[:], ident_l[:])
            # pixel-major bf16, one per pair slot; band3 cols are ONES for the
            # counts -- memset once, never overwritten (evacs skip them)
            xpix = [singles.tile([P, groups * P], BF16, tag=f"xpix{i}",
                                 name=f"xpix{i}") for i in range(2)]
            for xp in xpix:
                ones_v = bass.AP(
                    tensor=xp[:].tensor, offset=xp[:].offset + D * J,
                    ap=[xp[:].ap[0], [P, groups], [1, J]])
                nc.vector.memset(ones_v, 1.0)
            onesrow = singles.tile([1, P], F32R, tag="onesrow")
            onesrow_l = singles.tile([1, P], F32, tag="onesrow_l")
            nc.vector.memset(onesrow_l[:], 1.0)
            nc.vector.tensor_copy(onesrow[:], onesrow_l[:])

            # per-image-slot state (a/b)
            def slot_state(s):
                st = {}
                st["cent"] = singles.tile([K, D], F32, tag=f"cent{s}",
                                          name=f"cent{s}")
                st["wdiag"] = singles.tile([P, FD], F32R, tag=f"wdiag{s}",
                                           name=f"wdiag{s}")
                st["wrep"] = singles.tile([P, K], F32, tag=f"wrep{s}",
                                          name=f"wrep{s}")
                st["wt"] = singles.tile([4, K], F32, tag=f"wt{s}",
                                        name=f"wt{s}")
                st["w8"] = singles.tile([K, 4], F32, tag=f"w8{s}",
                                        name=f"w8{s}")
                nc.vector.memset(st["w8"][:, 3:4], 1.0)
                st["sq"] = singles.tile([K, D], F32, tag=f"sq{s}",
                                        name=f"sq{s}")
                st["csqc"] = singles.tile([K, 1], F32, tag=f"csqc{s}",
                                          name=f"csqc{s}")
                st["csqr"] = singles.tile([1, 2 * FD], F32R,
                                          tag=f"csqr{s}", name=f"csqr{s}")
                st["cntc"] = singles.tile([K, 1], F32, tag=f"cntc{s}",
                                          name=f"cntc{s}")
                st["recip"] = singles.tile([K, 1], F32, tag=f"recip{s}",
                                           name=f"recip{s}")
                st["pos"] = singles.tile([K, 1], F32, tag=f"pos{s}",
                                         name=f"pos{s}")
                st["cmean"] = singles.tile([K, D], F32, tag=f"cmean{s}",
                                           name=f"cmean{s}")
                st["ext"] = singles.tile([P, K], F32, tag=f"ext{s}",
                                         name=f"ext{s}")
                st["prod"] = singles.tile([P, FD], F32, tag=f"prod{s}",
                                          name=f"prod{s}")
                return st

            sts = [slot_state("a"), slot_state("b")]

            NQ = 8  # DMA queues per image load (parallel DMA bandwidth)

            def dma_image(img, buf):
                """Load image img into xT[buf] (128 bands, f32->f32r cast)."""
                w = groups * P // NQ
                for q in range(NQ):
                    nc.gpsimd.dma_start(
                        out=xT[buf][:, q * w:(q + 1) * w],
                        in_=xpl[img][:, q * w:(q + 1) * w])

            def prologue_quad(buf, slot, gq):
                """Build 4 groups of xpix[slot] from xT[buf] via f32r
                transposes; evacs alternate Act/DVE (f32r->bf16 cast)."""
                if True:
                    tp = bigpsum.tile([P, 4 * D * J], F32R, tag="big",
                                      name="tp")
                    for t in range(4):
                        g = gq * 4 + t
                        nc.tensor.transpose(
                            tp[:, t * D * J:(t + 1) * D * J],
                            xT[buf][0:D * J, g * P:(g + 1) * P],
                            identr[0:D * J, 0:D * J],
                        )
                    # strided evac: write bands 0:96 of each group, skip the
                    # persistent ones band
                    out_v = bass.AP(
                        tensor=xpix[slot][:].tensor,
                        offset=xpix[slot][:].offset + gq * 4 * P,
                        ap=[xpix[slot][:].ap[0], [P, 4], [1, D * J]])
                    in_v = bass.AP(
                        tensor=tp[:].tensor, offset=tp[:].offset,
                        ap=[tp[:].ap[0], [D * J, 4], [1, D * J]]).bitcast(F32)
                    if gq % 2 == 0:
                        nc.vector.tensor_copy(out_v, in_v)
                    else:
                        nc.scalar.copy(out_v, in_v)

            def w_chain(st):
                """centroids -> wdiag (f32r) + csqr (|c_k|^2 row)."""
                nc.scalar.mul(st["w8"][:, 0:D], st["cent"][:], -2.0)
                nc.scalar.activation(
                    st["sq"][:], st["cent"][:],
                    mybir.ActivationFunctionType.Square,
                    accum_out=st["csqc"][:])
                wtP = smallps.tile([4, K], F32, tag="small", name="wtP")
                nc.tensor.transpose(wtP[:], st["w8"][:], ident[0:K, 0:K])
                nc.scalar.copy(st["wt"][:], wtP[:])
                csqP = smallps.tile([1, K], F32, tag="small", name="csqP")
                nc.tensor.transpose(csqP[:], st["csqc"][:], ident[0:K, 0:K])
                csqt = smallsb.tile([1, K], F32, tag="csqt", name="csqt")
                nc.scalar.copy(csqt[:], csqP[:])
                csqt_b = bass.AP(
                    tensor=csqt[:].tensor, offset=csqt[:].offset,
                    ap=[csqt[:].ap[0], [0, 2], [1, K], [0, J]])
                csqr_v = bass.AP(
                    tensor=st["csqr"][:].tensor, offset=st["csqr"][:].offset,
                    ap=[st["csqr"][:].ap[0], [FD, 2], [J, K], [1, J]])
                nc.scalar.copy(csqr_v, csqt_b)
                # wrep[(c,j), k] = wt[c, k] via a PE broadcast matmul on the
                # f32 path (bc4 is a 0/1 selector); staged through Act copy
                wrepP = smallps.tile([P, K], F32, tag="small", name="wrepP")
                nc.tensor.matmul(
                    wrepP[:], bc4[:], st["wt"][:], start=True, stop=True)
                nc.scalar.copy(st["wrep"][:], wrepP[:])
                wrep_b = bass.AP(
                    tensor=st["wrep"][:].tensor, offset=st["wrep"][:].offset,
                    ap=[st["wrep"][:].ap[0], [1, K], [0, J]])
                nc.vector.tensor_tensor(
                    st["wdiag"][:].rearrange("p (k j) -> p k j", j=J),
                    diag01[:].rearrange("p (k j) -> p k j", j=J),
                    wrep_b, mybir.AluOpType.mult)

            def fold_update(st, seg):
                """seg PSUM -> S -> centroid update."""
                nc.vector.tensor_tensor(
                    st["prod"][:], seg[:], diag01[:], mybir.AluOpType.mult)
                nc.vector.tensor_reduce(
                    st["ext"][:],
                    st["prod"][:].rearrange("p (k j) -> p k j", j=J),
                    axis=mybir.AxisListType.X,
                    op=mybir.AluOpType.add)
                S = smallps.tile([K, 4], F32, tag="small", name="S")
                nc.tensor.matmul(S[:], st["ext"][:], csel[:],
                                 start=True, stop=True)
                nc.vector.tensor_scalar_max(st["cntc"][:], S[:, 3:4], 1.0)
                nc.vector.reciprocal(st["recip"][:], st["cntc"][:])
                nc.scalar.activation(
                    st["cmean"][:], S[:, 0:D],
                    mybir.ActivationFunctionType.Copy, scale=st["recip"][:])
                nc.vector.tensor_scalar(
                    st["pos"][:], S[:, 3:4], 0.5, None,
                    op0=mybir.AluOpType.is_ge)
                pos_b = bass.AP(
                    tensor=st["pos"][:].tensor, offset=st["pos"][:].offset,
                    ap=[st["pos"][:].ap[0], [0, D]])
                cdel = singles.tile([K, D], F32, tag="cdel", name="cdel")
                nc.vector.tensor_sub(cdel[:], st["cmean"][:], st["cent"][:])
                nc.vector.tensor_tensor(
                    cdel[:], cdel[:], pos_b, mybir.AluOpType.mult)
                nc.vector.tensor_add(st["cent"][:], st["cent"][:], cdel[:])

            # bc4 selector const (0/1): bc4[c, (c',j)] = [c'==c]
            bc4_d = nc.dram_tensor("bc4", [4, P], F32, kind="ExternalInput")
            bc4_l = singles.tile([4, P], F32, tag="bc4_l")
            nc.sync.dma_start(out=bc4_l[:], in_=bc4_d[:])
            bc4 = singles.tile([4, P], F32, tag="bc4")
            nc.vector.tensor_copy(bc4[:], bc4_l[:])

            def batch(st, buf, gq, pending):
                """One 4-group batch: scores -> conv -> min-tree -> mask.

                Returns the mask tile for the pending segsum emission."""
                sp = bigpsum.tile([P, GBATCH * FD], F32, tag="big", name="sp")
                for bank in range(2):
                    for tt in range(2):
                        t = bank * 2 + tt
                        g = gq * GBATCH + t
                        sl = sp[:, t * FD:(t + 1) * FD]
                        # tt==0 (start=True) clears the whole 2KB PSUM zero
                        # region, so tt==1 writes fresh with start=False; the
                        # bank-wide |c_k|^2 rank-1 update closes both groups
                        nc.tensor.matmul(
                            sl, xT[buf][:, g * P:(g + 1) * P],
                            st["wdiag"][:], start=(tt == 0), stop=False,
                            skip_group_check=True)
                    nc.tensor.matmul(
                        sp[:, bank * 2 * FD:(bank + 1) * 2 * FD],
                        onesrow[:], st["csqr"][:],
                        start=False, stop=True, skip_group_check=True)
                    if bank == 0 and pending is not None:
                        pending()
                # scores f32 PSUM -> fp16 SBUF on the Activation engine
                s16 = s16pool.tile([P, GBATCH * FD], F16, tag="s16",
                                   name="s16")
                nc.scalar.copy(s16[:], sp[:])

                def v(tl, tstr, kcnt, koff):
                    return bass.AP(
                        tensor=tl[:].tensor,
                        offset=tl[:].offset + koff * J,
                        ap=[tl[:].ap[0], [tstr, GBATCH], [J, kcnt], [1, J]])

                m1 = mvpool.tile([P, GBATCH * 4 * J], F16, tag="m1",
                                 name="m1")
                nc.vector.tensor_tensor(
                    v(m1, 4 * J, 4, 0),
                    v(s16, FD, 4, 0), v(s16, FD, 4, 4),
                    mybir.AluOpType.min)
                m2 = mvpool.tile([P, GBATCH * 2 * J], F16, tag="m2",
                                 name="m2")
                nc.vector.tensor_tensor(
                    v(m2, 2 * J, 2, 0),
                    v(m1, 4 * J, 2, 0), v(m1, 4 * J, 2, 2),
                    mybir.AluOpType.min)
                m3 = mvpool.tile([P, GBATCH * J], F16, tag="m3", name="m3")
                nc.vector.tensor_tensor(
                    v(m3, J, 1, 0),
                    v(m2, 2 * J, 1, 0), v(m2, 2 * J, 1, 1),
                    mybir.AluOpType.min)
                mk = maskpool.tile([P, GBATCH * FD], BF16, tag="mk",
                                   name="mk")
                m3_b = bass.AP(
                    tensor=m3[:].tensor, offset=m3[:].offset,
                    ap=[m3[:].ap[0], [J, GBATCH], [0, K], [1, J]])
                nc.vector.tensor_tensor(
                    v(mk, FD, K, 0), v(s16, FD, K, 0),
                    m3_b, mybir.AluOpType.is_le)
                return mk

            npairs = n_img // 2
            dma_image(0, 0)
            dma_image(1, 1)

            for pair in range(npairs):
                bufA = (2 * pair) % NBUF
                bufB = (2 * pair + 1) % NBUF
                stA, stB = sts[0], sts[1]

                # dummy PE ops absorb the image-DMA waits into the PE vector
                # clock (f32r matmuls have a single sync-wait slot)
                dummy = dumpps.tile([32, 32], F32, tag="dummy", name="dummy")
                nc.tensor.matmul(dummy[:], xT[bufA][0:32, 0:32],
                                 xT[bufA][0:32, 0:32], start=True, stop=True)
                nc.tensor.matmul(dummy[:], xT[bufB][0:32, 0:32],
                                 xT[bufB][0:32, 0:32], start=True, stop=True)

                if pair + 1 < npairs:
                    dma_image(2 * pair + 2, (2 * pair + 2) % NBUF)
                    dma_image(2 * pair + 3, (2 * pair + 3) % NBUF)

                nc.sync.dma_start(out=stA["cent"][:], in_=c0[2 * pair])
                nc.sync.dma_start(out=stB["cent"][:], in_=c0[2 * pair + 1])

                for q4 in range(groups // 4):
                    prologue_quad(bufA, 0, q4)
                    prologue_quad(bufB, 1, q4)

                def emit_seg(mk, gq, slot, seg):
                    for t in range(GBATCH):
                        g = gq * GBATCH + t
                        nc.tensor.matmul(
                            seg[:],
                            xpix[slot][:, g * P:(g + 1) * P],
                            mk[:, t * FD:(t + 1) * FD],
                            start=(g == 0), stop=(g == groups - 1),
                            skip_group_check=True)

                nbatch = groups // GBATCH
                for it in range(iters):
                    w_chain(stA)
                    w_chain(stB)
                    segA = segpsum.tile([P, FD], F32, tag="segA", name="segA")
                    segB = segpsum.tile([P, FD], F32, tag="segB", name="segB")
                    pend = [None, None]
                    for gq in range(nbatch):
                        mkA = batch(stA, bufA, gq, pend[0])
                        pend[0] = (lambda mk=mkA, gq=gq:
                                   emit_seg(mk, gq, 0, segA))
                        mkB = batch(stB, bufB, gq, pend[1])
                        pend[1] = (lambda mk=mkB, gq=gq:
                                   emit_seg(mk, gq, 1, segB))
                    pend[0]()
                    pend[1]()
                    fold_update(stA, segA)
                    fold_update(stB, segB)

                nc.sync.dma_start(out=out_d[2 * pair], in_=stA["cent"][:])
                nc.sync.dma_start(out=out_d[2 * pair + 1], in_=stB["cent"][:])

    nc.finalize()
    return nc


# ----------------------------------------------------------------------------
# host-side constants


def host_layout(pixels):
    """[B, n_pix, 3] -> xT DRAM layout [B, 4*J, n_pix//J]:
    out[b, c*J+j, g*P+p] = pixels4[b, g*J*P + j*P + p, c]
    where pixels4 = [x0, x1, x2, |x|^2]."""
    b, n_pix, _ = pixels.shape
    g = n_pix // (J * P)
    sq = (pixels.astype(np.float32) ** 2).sum(axis=2, keepdims=True)
    p4 = np.concatenate([pixels, sq], axis=2)  # [b, n_pix, 4]
    v = p4.reshape(b, g, J, P, 4).transpose(0, 4, 2, 1, 3)  # b c j g p
    return np.ascontiguousarray(v.reshape(b, 4 * J, g * P))


def host_constants():
    # k-major score columns: col = k*J + j
    diag01 = np.zeros((P, FD), np.float32)
    csel = np.zeros((P, 4), np.float32)
    bc4 = np.zeros((4, P), np.float32)
    for c in range(4):
        for j in range(J):
            q = c * J + j
            for k in range(K):
                diag01[q, k * J + j] = 1.0
            csel[q, c] = 1.0
            bc4[c, q] = 1.0
    ident = np.eye(P, dtype=np.float32)
    return diag01, csel, bc4, ident


_NC_CACHE = {}
TRACE = False
LAST_RESULTS = None


def _get_nc(n_img, iters, groups):
    key = (n_img, iters, groups)
    if key not in _NC_CACHE:
        _NC_CACHE[key] = build_kernel(n_img, iters, groups)
    return _NC_CACHE[key]


def kernel(inputs: np.ndarray) -> np.ndarray:
    x = np.ascontiguousarray(np.asarray(inputs, dtype=np.float32))
    assert x.shape == (B, H, W, D), x.shape
    pixels = x.reshape(B, N, D)

    # initial centroids (threefry permutation, stock-jax semantics)
    perm8 = np.array(PERM8, dtype=np.int64)             # [B, K]
    cent0 = np.take_along_axis(
        pixels, perm8[:, :, None].repeat(D, axis=2), axis=1
    ).astype(np.float32)                               # [B, K, D]

    planar = host_layout(pixels)  # [B, 3*J, N//J]

    diag01, csel, bc4, ident = host_constants()
    nc = _get_nc(IMG_PER_CORE, ITERS, GROUPS)

    in_maps = []
    for c in range(NCORES):
        sl = slice(c * IMG_PER_CORE, (c + 1) * IMG_PER_CORE)
        in_maps.append({
            "xplanar": np.ascontiguousarray(planar[sl]),
            "cent0": np.ascontiguousarray(cent0[sl]),
            "diag01": diag01,
            "csel": csel,
            "bc4": bc4,
            "ident": ident,
        })

    global LAST_RESULTS
    try:
        res = run_bass_kernel_spmd(nc, in_maps, core_ids=list(range(NCORES)),
                                   trace=TRACE)
    except Exception:
        if not TRACE:
            raise
        # tracing unsupported in this environment; rerun without
        res = run_bass_kernel_spmd(nc, in_maps, core_ids=list(range(NCORES)))
    LAST_RESULTS = res
    outs = [r["cent_out"].reshape(IMG_PER_CORE, K * D) for r in res.results]
    return np.concatenate(outs, axis=0).astype(np.float32)


if __name__ == "__main__":
    rs = np.random.RandomState(0)
    x = rs.random_sample((B, H, W, D)).astype(np.float32)
    out = kernel(inputs=x)
    print("out shape", out.shape, out.dtype)
    print(out[0])
